# revision 9
# baseline (speedup 1.0000x reference)
"""GCN encoder v3: bank-cell packed edge streams.

Δ vs kernel.py (v2): the edge stream is packed per (bank = 4 dst blocks,
chunk) cell instead of per (block, chunk), cutting the 128-rounding padding
from ~20% to ~6% of gather descriptors. Columns that straddle a block
boundary get one one-hot matmul per claiming block (host-scheduled, shared
across cores; per-core S masks zero out foreign edges).
"""
import numpy as np
import ml_dtypes
import concourse.bacc as bacc
import concourse.tile as tile
import concourse.bass as bass
import concourse.mybir as mybir
import concourse.bass_utils as bass_utils

N_CORES = 8
N_NODES = 100000
IN_C = 128
HID = 128
OUT_C = 64
R = N_NODES // N_CORES          # 12500 rows per core
NU = 4                          # src chunks / AllGather units
UR = R // NU                    # 3125 rows per unit per core
CHUNK = N_NODES // NU           # 25000 rows per (permuted) chunk
NBLK = (R + 127) // 128         # 98 dst blocks per core
BANK = 4                        # dst blocks per PSUM bank / stream cell
NBANK = (NBLK + BANK - 1) // BANK   # 25 banks
GCOLS = 8                       # max 128-edge cols per dma_gather (1024 idxs HW cap)
SMAX = 12                       # max one-hot claims per gather call

F32 = mybir.dt.float32
BF16 = mybir.dt.bfloat16
I16 = mybir.dt.int16
BF = ml_dtypes.bfloat16


def _wrap16(idx):
    n = idx.shape[0]
    a = idx.astype(np.int16).reshape(n // 16, 16).T
    return np.ascontiguousarray(np.tile(a, (8, 1)))


def _prep(edge_index):
    """Host prep: bank-cell packed per-core streams + shared claim schedule."""
    src = np.asarray(edge_index[0], dtype=np.int64)
    dst = np.asarray(edge_index[1], dtype=np.int64)
    deg = (np.bincount(dst, minlength=N_NODES) + 1).astype(np.float64)
    dinv = (1.0 / np.sqrt(deg)).astype(np.float32)

    k_of = src // R
    r_of = src % R
    psrc = CHUNK * (r_of // UR) + UR * k_of + (r_of % UR)
    c_of = psrc // CHUNK
    ci_of = psrc % CHUNK

    kd = dst // R
    ld = dst % R
    b_of = ld // 128
    dloc = ld % 128
    bank_of = b_of // BANK

    # cell order: stripes of 4 banks, chunk-major inside a stripe -- chunk c's
    # cells run consecutively (long same-table gather runs) and early stripes
    # only need AllGather unit c when they reach its pass (no head-of-line
    # stall on late units)
    cells = []
    for S in range((NBANK + 3) // 4):
        for c in range(NU):
            for B in range(4 * S, min(4 * S + 4, NBANK)):
                cells.append((B, c))
    rank_of = {bc: i for i, bc in enumerate(cells)}
    rank_lut = np.zeros((NBANK, NU), dtype=np.int64)
    for (B, c), i in rank_of.items():
        rank_lut[B, c] = i

    # stream order: (core, cell rank, block)
    order_key = (kd * len(cells) + rank_lut[bank_of, c_of]) * NBLK + b_of
    order = np.argsort(order_key, kind="stable")
    c_s, ci_s, kd_s, dloc_s, bank_s, b_s = (
        a[order] for a in (c_of, ci_of, kd, dloc, bank_of, b_of))
    rank_s = rank_lut[bank_s, c_s]

    cellcnt = np.zeros((N_CORES, NBANK, NU), dtype=np.int64)
    np.add.at(cellcnt, (kd_s, bank_s, c_s), 1)
    wcell = (cellcnt.max(axis=0) + 127) // 128        # [NBANK, NU] cols per cell

    blkcnt = np.zeros((N_CORES, NBANK, NU, BANK), dtype=np.int64)
    bslot = b_s - bank_s * BANK
    np.add.at(blkcnt, (kd_s, bank_s, c_s, bslot), 1)
    blkstart = np.cumsum(blkcnt, axis=3) - blkcnt      # exclusive prefix in cell
    blkend = blkstart + blkcnt

    # shared cell layout (in cell order)
    cellbase = {}
    col = 0
    for (B, c) in cells:
        if wcell[B, c] > 0:
            cellbase[(B, c)] = col
            col += int(wcell[B, c])
    LT = col
    L = LT * 128

    # shared claims per column (block slots whose [min_start, max_end) range
    # across cores touches the column)
    claims_blocks = [[] for _ in range(LT)]   # col -> [t_rel]
    for (B, c) in cells:
            if (B, c) not in cellbase:
                continue
            base = cellbase[(B, c)]
            for t in range(min(BANK, NBLK - B * BANK)):
                smin = int(blkstart[:, B, c, t].min())
                emax = int(blkend[:, B, c, t].max())
                if smin == emax:
                    continue
                for cc in range(base + smin // 128, base + (emax + 127) // 128):
                    claims_blocks[cc].append(t)
    # sequential s_idx in (column, claim) order
    claims_by_col = []
    s_counter = 0
    for cc in range(LT):
        lst = []
        for t in claims_blocks[cc]:
            lst.append((t, s_counter))
            s_counter += 1
        claims_by_col.append(lst)
    NS = s_counter

    # bank column ranges + first/last claim s_idx per bank
    bank_cols = {}
    for (B, c), base in cellbase.items():
        w = int(wcell[B, c])
        lo, hi = bank_cols.get(B, (10 ** 9, -1))
        bank_cols[B] = (min(lo, base), max(hi, base + w - 1))
    bank_first_s = {}
    bank_last_s = {}
    for B, (clo, chi) in bank_cols.items():
        ss = [s for cc in range(clo, chi + 1) for (_t, s) in claims_by_col[cc]]
        bank_first_s[B] = min(ss)
        bank_last_s[B] = max(ss)

    # gather calls: pack each consecutive same-chunk run of cells into
    # <= GCOLS-col pieces (cells of one (stripe, chunk) pass are adjacent)
    calls = []
    i = 0
    while i < len(cells):
        B0, c0c = cells[i]
        if (B0, c0c) not in cellbase:
            i += 1
            continue
        base = cellbase[(B0, c0c)]
        end = base
        while i < len(cells) and cells[i][1] == c0c and \
                cells[i] in cellbase and cellbase[cells[i]] == end:
            end += int(wcell[cells[i][0], cells[i][1]])
            i += 1
        p = base
        while p < end:
            n = min(GCOLS, end - p)
            calls.append((c0c, p, n))
            p += n

    bank_of_col = np.full(LT, -1, dtype=np.int64)
    for (B, c), base in cellbase.items():
        bank_of_col[base:base + int(wcell[B, c])] = B

    # per-core streams
    per_core = []
    for k in range(N_CORES):
        idx_arr = np.zeros(L, dtype=np.int16)
        dst_arr = np.full((NS, 128), -1.0, dtype=np.float32)
        sel = kd_s == k
        ci_k = ci_s[sel]
        c_k = c_s[sel]
        dl_k = dloc_s[sel]
        bank_k = bank_s[sel]
        b_k = b_s[sel]
        key_k = rank_lut[bank_k, c_k] * NBLK + b_k
        for (B, c), base in cellbase.items():
            rnk = rank_of[(B, c)]
            lo = np.searchsorted(key_k, rnk * NBLK)
            hi = np.searchsorted(key_k, rnk * NBLK + NBLK)
            n = hi - lo
            if n:
                idx_arr[base * 128: base * 128 + n] = ci_k[lo:hi]
            for cc in range(base, base + int(wcell[B, c])):
                p0 = lo + (cc - base) * 128
                for (t, s_idx) in claims_by_col[cc]:
                    s0 = lo + int(blkstart[k, B, c, t])
                    e0 = lo + int(blkend[k, B, c, t])
                    a0, a1 = max(s0, p0), min(e0, p0 + 128)
                    if a0 < a1:
                        dst_arr[s_idx, a0 - p0:a1 - p0] = dl_k[a0:a1]
        idx16 = _wrap16(idx_arr)
        dst128 = np.ascontiguousarray(dst_arr.T.astype(BF))   # [128, NS]
        per_core.append((idx16, dst128))

    plan = dict(calls=calls, LT=LT, L=L, NS=NS,
                claims_by_col=claims_by_col, bank_of_col=bank_of_col,
                bank_first_s=bank_first_s, bank_last_s=bank_last_s)
    return dinv, plan, per_core


def _build(plan, reps=1, nocoll=False, cfg=None):
    cfg = cfg or {}
    GBUFS = cfg.get("gbufs", 10)
    SBUFS = cfg.get("sbufs", 8)
    nc = bacc.Bacc("TRN2", target_bir_lowering=False, debug=False, num_devices=N_CORES,
                   num_swdge_queues=4,
                   dynamic_dma_scratch_size=cfg.get("dma_scratch", 32768))
    LT, L, NS = plan["LT"], plan["L"], plan["NS"]
    calls = plan["calls"]
    claims_by_col = plan["claims_by_col"]
    bank_of_col = plan["bank_of_col"]
    bank_first_s, bank_last_s = plan["bank_first_s"], plan["bank_last_s"]
    XCOLS = NBLK * 128

    xT = nc.dram_tensor("xT", [128, XCOLS], BF16, kind="ExternalInput")
    idx_in = nc.dram_tensor("idx_in", [128, L // 16], I16, kind="ExternalInput")
    s_in = nc.dram_tensor("s_in", [128, NS, 128], BF16, kind="ExternalInput")
    dinv_in = nc.dram_tensor("dinv_in", [128, NBLK], F32, kind="ExternalInput")
    ident_in = nc.dram_tensor("ident_in", [128, 128], BF16, kind="ExternalInput")
    w1_in = nc.dram_tensor("w1_in", [128, HID], BF16, kind="ExternalInput")
    wmu_in = nc.dram_tensor("wmu_in", [HID, OUT_C], BF16, kind="ExternalInput")
    wls_in = nc.dram_tensor("wls_in", [HID, OUT_C], BF16, kind="ExternalInput")
    b1_in = nc.dram_tensor("b1_in", [128, HID], F32, kind="ExternalInput")
    bmuls_in = nc.dram_tensor("bmuls_in", [128, 2 * OUT_C], F32, kind="ExternalInput")
    mu_out = nc.dram_tensor("mu_out", [128, NBLK * OUT_C], BF16, kind="ExternalOutput")
    ls_out = nc.dram_tensor("ls_out", [128, NBLK * OUT_C], BF16, kind="ExternalOutput")

    with tile.TileContext(nc) as tc:
        with (
            tc.tile_pool(name="const", bufs=1) as cpool,
            tc.tile_pool(name="yh", bufs=1) as yhp,
            tc.tile_pool(name="gat", bufs=GBUFS) as gp,
            tc.tile_pool(name="sel", bufs=SBUFS) as sp,
            tc.tile_pool(name="epi", bufs=4) as ep,
            tc.tile_pool(name="psA", bufs=4, space="PSUM") as psA,
            tc.tile_pool(name="psB", bufs=2, space="PSUM") as psB,
            tc.tile_pool(name="dram", bufs=1, space="DRAM") as dram,
        ):
            idx_sb = cpool.tile([128, L // 16], I16)
            dinv_sb = cpool.tile([128, NBLK], F32)
            ident_sb = cpool.tile([128, 128], BF16)
            w1_sb = cpool.tile([128, HID], BF16)
            wmu_sb = cpool.tile([HID, OUT_C], BF16)
            wls_sb = cpool.tile([HID, OUT_C], BF16)
            b1_sb = cpool.tile([128, HID], F32)
            bmuls_sb = cpool.tile([128, 2 * OUT_C], F32)
            for sb, dr in ((idx_sb, idx_in), (dinv_sb, dinv_in),
                           (ident_sb, ident_in), (w1_sb, w1_in),
                           (wmu_sb, wmu_in), (wls_sb, wls_in), (b1_sb, b1_in),
                           (bmuls_sb, bmuls_in)):
                nc.sync.dma_start(out=sb[:], in_=dr.ap()[:])

            x_all = yhp.tile([128, XCOLS], BF16, tag="xbig", name="x_all")
            nc.sync.dma_start(out=x_all[:], in_=xT.ap()[:])
            y_all = cpool.tile([128, NBLK, 128], BF16, name="y_all")
            h_all = cpool.tile([128, NBLK, 128], BF16, name="h_all")
            mu_all = cpool.tile([128, NBLK * OUT_C], BF16, name="mu_all")
            ls_all = cpool.tile([128, NBLK * OUT_C], BF16, name="ls_all")

            y_in = dram.tile([R, HID], BF16)
            h_in = dram.tile([R, HID], BF16)

            def alloc_full(pfx):
                return [dram.tile([CHUNK, HID], BF16, addr_space="Shared",
                                  tag=f"{pfx}{j}", name=f"{pfx}{j}")
                        for j in range(NU)]

            unit_last_block = [(UR * (j + 1) - 1) // 128 for j in range(NU)]

            def write_unit_rows(dram_t, sb_all, j):
                lo, hi = UR * j, UR * (j + 1)
                b0, p0 = lo // 128, lo % 128
                b1, p1 = hi // 128, hi % 128
                if p0:
                    n = 128 - p0
                    nc.sync.dma_start(out=dram_t[lo:lo + n, :],
                                      in_=sb_all[p0:p0 + n, b0, :])
                    lo += n
                    b0 += 1
                if b0 < b1:
                    nc.sync.dma_start(
                        out=dram_t[128 * b0:128 * b1, :].rearrange(
                            "(b p) f -> p b f", p=128),
                        in_=sb_all[:, b0:b1, :])
                if p1:
                    nc.sync.dma_start(out=dram_t[128 * b1:128 * b1 + p1, :],
                                      in_=sb_all[0:p1, b1, :])

            def ag_unit(t_in, t_full, j):
                if nocoll:
                    nc.sync.dma_start(out=t_full[j][0:UR, :],
                                      in_=t_in[UR * j:UR * (j + 1), :])
                else:
                    nc.gpsimd.collective_compute(
                        "AllGather", mybir.AluOpType.bypass,
                        replica_groups=[list(range(N_CORES))],
                        ins=[t_in[UR * j:UR * (j + 1), :]],
                        outs=[t_full[j][:]],
                    )

            def phase1(y_full):
                nxt = 0
                for b in range(NBLK):
                    y_ps = psB.tile([128, HID], F32, tag="outps", space="PSUM",
                                    name="y_ps")
                    nc.tensor.matmul(out=y_ps[:], lhsT=x_all[:, 128 * b:128 * (b + 1)],
                                     rhs=w1_sb[:], start=True, stop=True)
                    nc.scalar.activation(out=y_all[:, b, :], in_=y_ps[:],
                                         func=mybir.ActivationFunctionType.Copy,
                                         scale=dinv_sb[:, b:b + 1])
                    while nxt < NU and b == unit_last_block[nxt]:
                        write_unit_rows(y_in, y_all, nxt)
                        ag_unit(y_in, y_full, nxt)
                        nxt += 1

            def aggregation(table, epilogue):
                acc = {}
                done = [0]
                for ci, (c, c0, ncols) in enumerate(calls):
                    g_tile = gp.tile([128, GCOLS, 128], BF16, tag="g")
                    nc.gpsimd.dma_gather(
                        out_ap=g_tile[:, 0:ncols, :],
                        in_ap=table[c][:],
                        idxs_ap=idx_sb[:, 8 * c0: 8 * (c0 + ncols)],
                        num_idxs=128 * ncols, num_idxs_reg=128 * ncols,
                        elem_size=128,
                        queue_num=ci % 4,
                        single_packet=True,
                    )
                    cl = [(t, s_idx, cc) for cc in range(c0, c0 + ncols)
                          for (t, s_idx) in claims_by_col[cc]]
                    ns = len(cl)
                    assert 0 < ns <= SMAX, (ns, ncols)
                    s0 = cl[0][1]
                    assert [s for (_t, s, _c) in cl] == list(range(s0, s0 + ns))
                    s_tile = sp.tile([128, SMAX, 128], BF16, tag="s")
                    nc.sync.dma_start(out=s_tile[:, 0:ns, :],
                                      in_=s_in.ap()[:, s0:s0 + ns, :])
                    for si, (t, s_idx, cc) in enumerate(cl):
                        B = int(bank_of_col[cc])
                        if s_idx == bank_first_s[B]:
                            acc[B] = psA.tile([128, 512], F32, tag="acc",
                                              name="acc", space="PSUM")
                        a_ps = acc[B]
                        sl = slice(128 * t, 128 * (t + 1))
                        nc.tensor.matmul(out=a_ps[:, sl],
                                         lhsT=s_tile[:, si, :],
                                         rhs=g_tile[:, cc - c0, :],
                                         start=(s_idx == bank_first_s[B]),
                                         stop=(s_idx == bank_last_s[B]))
                        if s_idx == bank_last_s[B]:
                            for t2 in range(min(BANK, NBLK - B * BANK)):
                                epilogue(B * BANK + t2,
                                         a_ps[:, 128 * t2:128 * (t2 + 1)])
                                done[0] += 1
                            del acc[B]
                assert done[0] == NBLK, done[0]

            def make_epi1(h_full):
                state = {"nxt": 0}

                def epi1(b, acc_ap):
                    t1 = ep.tile([128, HID], F32, tag="t1")
                    nc.vector.tensor_tensor(out=t1[:], in0=acc_ap,
                                            in1=y_all[:, b, :],
                                            op=mybir.AluOpType.add)
                    nc.vector.tensor_scalar_mul(out=t1[:], in0=t1[:],
                                                scalar1=dinv_sb[:, b:b + 1])
                    nc.vector.tensor_tensor(out=t1[:], in0=t1[:], in1=b1_sb[:],
                                            op=mybir.AluOpType.add)
                    nc.scalar.activation(out=h_all[:, b, :], in_=t1[:],
                                         func=mybir.ActivationFunctionType.Relu,
                                         scale=dinv_sb[:, b:b + 1])
                    while (state["nxt"] < NU
                           and b == unit_last_block[state["nxt"]]):
                        write_unit_rows(h_in, h_all, state["nxt"])
                        ag_unit(h_in, h_full, state["nxt"])
                        state["nxt"] += 1
                return epi1

            def first_half():
                y_full = alloc_full("y_full")
                phase1(y_full)
                h_full = alloc_full("h_full")
                aggregation(y_full, make_epi1(h_full))
                return h_full

            def epi2(b, acc_ap):
                t2 = ep.tile([128, HID], F32, tag="t2")
                nc.vector.tensor_tensor(out=t2[:], in0=acc_ap, in1=h_all[:, b, :],
                                        op=mybir.AluOpType.add)
                g2_sb = ep.tile([128, HID], BF16, tag="g2")
                nc.scalar.activation(out=g2_sb[:], in_=t2[:],
                                     func=mybir.ActivationFunctionType.Copy,
                                     scale=dinv_sb[:, b:b + 1])
                tp_ps = psB.tile([128, HID], BF16, tag="tp", space="PSUM")
                nc.tensor.transpose(out=tp_ps[:], in_=g2_sb[:], identity=ident_sb[:])
                g2t_sb = ep.tile([128, HID], BF16, tag="g2t")
                nc.scalar.activation(out=g2t_sb[:], in_=tp_ps[:],
                                     func=mybir.ActivationFunctionType.Copy)
                o_ps = psB.tile([128, 2 * OUT_C], F32, tag="outps", space="PSUM")
                nc.tensor.matmul(out=o_ps[:, 0:OUT_C], lhsT=g2t_sb[:], rhs=wmu_sb[:],
                                 start=True, stop=True)
                nc.tensor.matmul(out=o_ps[:, OUT_C:2 * OUT_C], lhsT=g2t_sb[:],
                                 rhs=wls_sb[:], start=True, stop=True)
                nc.vector.tensor_tensor(out=mu_all[:, OUT_C * b:OUT_C * (b + 1)],
                                        in0=o_ps[:, 0:OUT_C],
                                        in1=bmuls_sb[:, 0:OUT_C],
                                        op=mybir.AluOpType.add)
                nc.vector.tensor_tensor(out=ls_all[:, OUT_C * b:OUT_C * (b + 1)],
                                        in0=o_ps[:, OUT_C:2 * OUT_C],
                                        in1=bmuls_sb[:, OUT_C:2 * OUT_C],
                                        op=mybir.AluOpType.add)

            for _ in range(reps):
                h_full = first_half()
                aggregation(h_full, epi2)
                nc.sync.dma_start(out=mu_out.ap()[:], in_=mu_all[:])
                nc.sync.dma_start(out=ls_out.ap()[:], in_=ls_all[:])

    nc.compile()
    return nc


TUNED_CFG = {"gbufs": 10, "sbufs": 8}


def build_in_maps(inputs, dinv, per_core):
    x = np.asarray(inputs["x"])
    ident = np.eye(128, dtype=np.float32).astype(BF)
    w1 = np.asarray(inputs["W1"], np.float32).astype(BF)
    wmu = np.asarray(inputs["W_mu"], np.float32).astype(BF)
    wls = np.asarray(inputs["W_ls"], np.float32).astype(BF)
    b1t = np.tile(np.asarray(inputs["b1"], np.float32), (128, 1))
    bmuls = np.tile(np.concatenate([np.asarray(inputs["b_mu"], np.float32),
                                    np.asarray(inputs["b_ls"], np.float32)]),
                    (128, 1))
    XCOLS = NBLK * 128
    in_maps = []
    for k in range(N_CORES):
        idx16, dst128 = per_core[k]
        xk = x[R * k:R * (k + 1)].astype(np.float32)
        xTk = np.zeros((128, XCOLS), dtype=BF)
        xTk[:, :R] = np.ascontiguousarray(xk.T).astype(BF)
        dv = dinv[R * k:R * (k + 1)]
        padded = np.ones(NBLK * 128, dtype=np.float32)
        padded[:R] = dv
        dinv_blk = np.ascontiguousarray(padded.reshape(NBLK, 128).T)
        # prebuilt one-hot S masks: [128 edge, NS, 128 dst] bf16
        dstm = dst128.T.astype(np.float32)            # [NS, 128]
        onehot = (dstm[:, :, None] == np.arange(128, dtype=np.float32)[None, None, :])
        s_full = np.ascontiguousarray(
            onehot.transpose(1, 0, 2)).astype(BF)     # [128, NS, 128]
        in_maps.append({
            "xT": xTk, "idx_in": idx16, "s_in": s_full, "dinv_in": dinv_blk,
            "ident_in": ident, "w1_in": w1, "wmu_in": wmu,
            "wls_in": wls, "b1_in": b1t, "bmuls_in": bmuls,
        })
    return in_maps


def _unstage(arr):
    a = np.asarray(arr).astype(np.float32).reshape(128, NBLK, OUT_C)
    a = a.transpose(1, 0, 2)
    return np.ascontiguousarray(a.reshape(NBLK * 128, OUT_C)[:R])


def kernel(x, edge_index, W1, b1, W_mu, b_mu, W_ls, b_ls):
    inputs = {"x": x, "W1": W1, "b1": b1, "W_mu": W_mu, "b_mu": b_mu,
              "W_ls": W_ls, "b_ls": b_ls}
    dinv, plan, per_core = _prep(np.asarray(edge_index))
    nc = _build(plan, cfg=TUNED_CFG)
    in_maps = build_in_maps(inputs, dinv, per_core)
    res = bass_utils.run_bass_kernel_spmd(nc, in_maps, core_ids=list(range(N_CORES)))
    mu = np.concatenate([_unstage(res.results[k]["mu_out"]) for k in range(N_CORES)],
                        axis=0)
    ls = np.concatenate([_unstage(res.results[k]["ls_out"]) for k in range(N_CORES)],
                        axis=0)
    return (mu, ls)


# revision 11
# speedup vs baseline: 1.0523x; 1.0523x over previous
"""GCN encoder v3: bank-cell packed edge streams.

Δ vs kernel.py (v2): the edge stream is packed per (bank = 4 dst blocks,
chunk) cell instead of per (block, chunk), cutting the 128-rounding padding
from ~20% to ~6% of gather descriptors. Columns that straddle a block
boundary get one one-hot matmul per claiming block (host-scheduled, shared
across cores; per-core S masks zero out foreign edges).
"""
import numpy as np
import ml_dtypes
import concourse.bacc as bacc
import concourse.tile as tile
import concourse.bass as bass
import concourse.mybir as mybir
import concourse.bass_utils as bass_utils

N_CORES = 8
N_NODES = 100000
IN_C = 128
HID = 128
OUT_C = 64
R = N_NODES // N_CORES          # 12500 rows per core
NU = 4                          # src chunks / AllGather units
UR = R // NU                    # 3125 rows per unit per core
CHUNK = N_NODES // NU           # 25000 rows per (permuted) chunk
NBLK = (R + 127) // 128         # 98 dst blocks per core
BANK = 4                        # dst blocks per PSUM bank / stream cell
NBANK = (NBLK + BANK - 1) // BANK   # 25 banks
GCOLS = 8                       # max 128-edge cols per dma_gather (1024 idxs HW cap)
SMAX = 12                       # max one-hot claims per gather call

F32 = mybir.dt.float32
BF16 = mybir.dt.bfloat16
I16 = mybir.dt.int16
BF = ml_dtypes.bfloat16


def _wrap16(idx):
    n = idx.shape[0]
    a = idx.astype(np.int16).reshape(n // 16, 16).T
    return np.ascontiguousarray(np.tile(a, (8, 1)))


def _prep(edge_index):
    """Host prep: bank-cell packed per-core streams + shared claim schedule."""
    src = np.asarray(edge_index[0], dtype=np.int64)
    dst = np.asarray(edge_index[1], dtype=np.int64)
    deg = (np.bincount(dst, minlength=N_NODES) + 1).astype(np.float64)
    dinv = (1.0 / np.sqrt(deg)).astype(np.float32)

    k_of = src // R
    r_of = src % R
    psrc = CHUNK * (r_of // UR) + UR * k_of + (r_of % UR)
    c_of = psrc // CHUNK
    ci_of = psrc % CHUNK

    kd = dst // R
    ld = dst % R
    b_of = ld // 128
    dloc = ld % 128
    bank_of = b_of // BANK

    # cell order: stripes of 4 banks, chunk-major inside a stripe -- chunk c's
    # cells run consecutively (long same-table gather runs) and early stripes
    # only need AllGather unit c when they reach its pass (no head-of-line
    # stall on late units)
    cells = []
    for S in range((NBANK + 3) // 4):
        for c in range(NU):
            for B in range(4 * S, min(4 * S + 4, NBANK)):
                cells.append((B, c))
    rank_of = {bc: i for i, bc in enumerate(cells)}
    rank_lut = np.zeros((NBANK, NU), dtype=np.int64)
    for (B, c), i in rank_of.items():
        rank_lut[B, c] = i

    # stream order: (core, cell rank, block)
    order_key = (kd * len(cells) + rank_lut[bank_of, c_of]) * NBLK + b_of
    order = np.argsort(order_key, kind="stable")
    c_s, ci_s, kd_s, dloc_s, bank_s, b_s = (
        a[order] for a in (c_of, ci_of, kd, dloc, bank_of, b_of))
    rank_s = rank_lut[bank_s, c_s]

    cellcnt = np.zeros((N_CORES, NBANK, NU), dtype=np.int64)
    np.add.at(cellcnt, (kd_s, bank_s, c_s), 1)
    wcell = (cellcnt.max(axis=0) + 127) // 128        # [NBANK, NU] cols per cell

    blkcnt = np.zeros((N_CORES, NBANK, NU, BANK), dtype=np.int64)
    bslot = b_s - bank_s * BANK
    np.add.at(blkcnt, (kd_s, bank_s, c_s, bslot), 1)
    blkstart = np.cumsum(blkcnt, axis=3) - blkcnt      # exclusive prefix in cell
    blkend = blkstart + blkcnt

    # shared cell layout (in cell order)
    cellbase = {}
    col = 0
    for (B, c) in cells:
        if wcell[B, c] > 0:
            cellbase[(B, c)] = col
            col += int(wcell[B, c])
    LT = col
    L = LT * 128

    # shared claims per column (block slots whose [min_start, max_end) range
    # across cores touches the column)
    claims_blocks = [[] for _ in range(LT)]   # col -> [t_rel]
    for (B, c) in cells:
            if (B, c) not in cellbase:
                continue
            base = cellbase[(B, c)]
            for t in range(min(BANK, NBLK - B * BANK)):
                smin = int(blkstart[:, B, c, t].min())
                emax = int(blkend[:, B, c, t].max())
                if smin == emax:
                    continue
                for cc in range(base + smin // 128, base + (emax + 127) // 128):
                    claims_blocks[cc].append(t)
    # sequential s_idx in (column, claim) order
    claims_by_col = []
    s_counter = 0
    for cc in range(LT):
        lst = []
        for t in claims_blocks[cc]:
            lst.append((t, s_counter))
            s_counter += 1
        claims_by_col.append(lst)
    NS = s_counter

    # bank column ranges + first/last claim s_idx per bank
    bank_cols = {}
    for (B, c), base in cellbase.items():
        w = int(wcell[B, c])
        lo, hi = bank_cols.get(B, (10 ** 9, -1))
        bank_cols[B] = (min(lo, base), max(hi, base + w - 1))
    bank_first_s = {}
    bank_last_s = {}
    for B, (clo, chi) in bank_cols.items():
        ss = [s for cc in range(clo, chi + 1) for (_t, s) in claims_by_col[cc]]
        bank_first_s[B] = min(ss)
        bank_last_s[B] = max(ss)

    # gather calls: pack each consecutive same-chunk run of cells into
    # <= GCOLS-col pieces (cells of one (stripe, chunk) pass are adjacent)
    calls = []
    i = 0
    while i < len(cells):
        B0, c0c = cells[i]
        if (B0, c0c) not in cellbase:
            i += 1
            continue
        base = cellbase[(B0, c0c)]
        end = base
        while i < len(cells) and cells[i][1] == c0c and \
                cells[i] in cellbase and cellbase[cells[i]] == end:
            end += int(wcell[cells[i][0], cells[i][1]])
            i += 1
        p = base
        while p < end:
            n = min(GCOLS, end - p)
            calls.append((c0c, p, n))
            p += n

    bank_of_col = np.full(LT, -1, dtype=np.int64)
    for (B, c), base in cellbase.items():
        bank_of_col[base:base + int(wcell[B, c])] = B

    # per-core streams
    per_core = []
    for k in range(N_CORES):
        idx_arr = np.zeros(L, dtype=np.int16)
        dst_arr = np.full((NS, 128), -1.0, dtype=np.float32)
        sel = kd_s == k
        ci_k = ci_s[sel]
        c_k = c_s[sel]
        dl_k = dloc_s[sel]
        bank_k = bank_s[sel]
        b_k = b_s[sel]
        key_k = rank_lut[bank_k, c_k] * NBLK + b_k
        for (B, c), base in cellbase.items():
            rnk = rank_of[(B, c)]
            lo = np.searchsorted(key_k, rnk * NBLK)
            hi = np.searchsorted(key_k, rnk * NBLK + NBLK)
            n = hi - lo
            if n:
                idx_arr[base * 128: base * 128 + n] = ci_k[lo:hi]
            for cc in range(base, base + int(wcell[B, c])):
                p0 = lo + (cc - base) * 128
                for (t, s_idx) in claims_by_col[cc]:
                    s0 = lo + int(blkstart[k, B, c, t])
                    e0 = lo + int(blkend[k, B, c, t])
                    a0, a1 = max(s0, p0), min(e0, p0 + 128)
                    if a0 < a1:
                        dst_arr[s_idx, a0 - p0:a1 - p0] = dl_k[a0:a1]
        idx16 = _wrap16(idx_arr)
        dst128 = np.ascontiguousarray(dst_arr.T.astype(BF))   # [128, NS]
        per_core.append((idx16, dst128))

    plan = dict(calls=calls, LT=LT, L=L, NS=NS,
                claims_by_col=claims_by_col, bank_of_col=bank_of_col,
                bank_first_s=bank_first_s, bank_last_s=bank_last_s)
    return dinv, plan, per_core


def _build(plan, reps=1, nocoll=False, cfg=None):
    cfg = cfg or {}
    GBUFS = cfg.get("gbufs", 10)
    SBUFS = cfg.get("sbufs", 8)
    nc = bacc.Bacc("TRN2", target_bir_lowering=False, debug=False, num_devices=N_CORES,
                   num_swdge_queues=4,
                   dynamic_dma_scratch_size=cfg.get("dma_scratch", 32768))
    LT, L, NS = plan["LT"], plan["L"], plan["NS"]
    calls = plan["calls"]
    claims_by_col = plan["claims_by_col"]
    bank_of_col = plan["bank_of_col"]
    bank_first_s, bank_last_s = plan["bank_first_s"], plan["bank_last_s"]
    XCOLS = NBLK * 128

    xT = nc.dram_tensor("xT", [128, XCOLS], BF16, kind="ExternalInput")
    idx_in = nc.dram_tensor("idx_in", [128, L // 16], I16, kind="ExternalInput")
    dst_in = nc.dram_tensor("dst_in", [128, NS], BF16, kind="ExternalInput")
    dinv_in = nc.dram_tensor("dinv_in", [128, NBLK], F32, kind="ExternalInput")
    iota_in = nc.dram_tensor("iota_in", [128, 128], BF16, kind="ExternalInput")
    ident_in = nc.dram_tensor("ident_in", [128, 128], BF16, kind="ExternalInput")
    w1_in = nc.dram_tensor("w1_in", [128, HID], BF16, kind="ExternalInput")
    wmu_in = nc.dram_tensor("wmu_in", [HID, OUT_C], BF16, kind="ExternalInput")
    wls_in = nc.dram_tensor("wls_in", [HID, OUT_C], BF16, kind="ExternalInput")
    b1_in = nc.dram_tensor("b1_in", [128, HID], F32, kind="ExternalInput")
    bmuls_in = nc.dram_tensor("bmuls_in", [128, 2 * OUT_C], F32, kind="ExternalInput")
    mu_out = nc.dram_tensor("mu_out", [128, NBLK * OUT_C], BF16, kind="ExternalOutput")
    ls_out = nc.dram_tensor("ls_out", [128, NBLK * OUT_C], BF16, kind="ExternalOutput")

    with tile.TileContext(nc) as tc:
        with (
            tc.tile_pool(name="const", bufs=1) as cpool,
            tc.tile_pool(name="yh", bufs=1) as yhp,
            tc.tile_pool(name="gat", bufs=GBUFS) as gp,
            tc.tile_pool(name="sel", bufs=SBUFS) as sp,
            tc.tile_pool(name="epi", bufs=4) as ep,
            tc.tile_pool(name="psA", bufs=4, space="PSUM") as psA,
            tc.tile_pool(name="psB", bufs=2, space="PSUM") as psB,
            tc.tile_pool(name="dram", bufs=1, space="DRAM") as dram,
        ):
            idx_sb = cpool.tile([128, L // 16], I16)
            dst_sb = cpool.tile([128, NS], BF16)
            dinv_sb = cpool.tile([128, NBLK], F32)
            iota_sb = cpool.tile([128, 128], BF16)
            ident_sb = cpool.tile([128, 128], BF16)
            w1_sb = cpool.tile([128, HID], BF16)
            wmu_sb = cpool.tile([HID, OUT_C], BF16)
            wls_sb = cpool.tile([HID, OUT_C], BF16)
            b1_sb = cpool.tile([128, HID], F32)
            bmuls_sb = cpool.tile([128, 2 * OUT_C], F32)
            for sb, dr in ((idx_sb, idx_in), (dst_sb, dst_in), (dinv_sb, dinv_in),
                           (iota_sb, iota_in), (ident_sb, ident_in), (w1_sb, w1_in),
                           (wmu_sb, wmu_in), (wls_sb, wls_in), (b1_sb, b1_in),
                           (bmuls_sb, bmuls_in)):
                nc.sync.dma_start(out=sb[:], in_=dr.ap()[:])

            x_all = yhp.tile([128, XCOLS], BF16, tag="xbig", name="x_all")
            nc.sync.dma_start(out=x_all[:], in_=xT.ap()[:])
            y_all = cpool.tile([128, NBLK, 128], BF16, name="y_all")
            h_all = cpool.tile([128, NBLK, 128], BF16, name="h_all")
            mu_all = cpool.tile([128, NBLK * OUT_C], BF16, name="mu_all")
            ls_all = cpool.tile([128, NBLK * OUT_C], BF16, name="ls_all")

            y_in = dram.tile([R, HID], BF16)
            h_in = dram.tile([R, HID], BF16)

            def alloc_full(pfx):
                return [dram.tile([CHUNK, HID], BF16, addr_space="Shared",
                                  tag=f"{pfx}{j}", name=f"{pfx}{j}")
                        for j in range(NU)]

            unit_last_block = [(UR * (j + 1) - 1) // 128 for j in range(NU)]

            def write_unit_rows(dram_t, sb_all, j):
                lo, hi = UR * j, UR * (j + 1)
                b0, p0 = lo // 128, lo % 128
                b1, p1 = hi // 128, hi % 128
                if p0:
                    n = 128 - p0
                    nc.sync.dma_start(out=dram_t[lo:lo + n, :],
                                      in_=sb_all[p0:p0 + n, b0, :])
                    lo += n
                    b0 += 1
                if b0 < b1:
                    nc.sync.dma_start(
                        out=dram_t[128 * b0:128 * b1, :].rearrange(
                            "(b p) f -> p b f", p=128),
                        in_=sb_all[:, b0:b1, :])
                if p1:
                    nc.sync.dma_start(out=dram_t[128 * b1:128 * b1 + p1, :],
                                      in_=sb_all[0:p1, b1, :])

            def ag_unit(t_in, t_full, j):
                if nocoll:
                    nc.sync.dma_start(out=t_full[j][0:UR, :],
                                      in_=t_in[UR * j:UR * (j + 1), :])
                else:
                    nc.gpsimd.collective_compute(
                        "AllGather", mybir.AluOpType.bypass,
                        replica_groups=[list(range(N_CORES))],
                        ins=[t_in[UR * j:UR * (j + 1), :]],
                        outs=[t_full[j][:]],
                    )

            def phase1(y_full):
                nxt = 0
                for b in range(NBLK):
                    y_ps = psB.tile([128, HID], F32, tag="outps", space="PSUM",
                                    name="y_ps")
                    nc.tensor.matmul(out=y_ps[:], lhsT=x_all[:, 128 * b:128 * (b + 1)],
                                     rhs=w1_sb[:], start=True, stop=True)
                    nc.scalar.activation(out=y_all[:, b, :], in_=y_ps[:],
                                         func=mybir.ActivationFunctionType.Copy,
                                         scale=dinv_sb[:, b:b + 1])
                    while nxt < NU and b == unit_last_block[nxt]:
                        write_unit_rows(y_in, y_all, nxt)
                        ag_unit(y_in, y_full, nxt)
                        nxt += 1

            def aggregation(table, epilogue):
                acc = {}
                done = [0]
                for ci, (c, c0, ncols) in enumerate(calls):
                    g_tile = gp.tile([128, GCOLS, 128], BF16, tag="g")
                    nc.gpsimd.dma_gather(
                        out_ap=g_tile[:, 0:ncols, :],
                        in_ap=table[c][:],
                        idxs_ap=idx_sb[:, 8 * c0: 8 * (c0 + ncols)],
                        num_idxs=128 * ncols, num_idxs_reg=128 * ncols,
                        elem_size=128,
                        queue_num=ci % 4,
                        single_packet=True,
                    )
                    cl = [(t, s_idx, cc) for cc in range(c0, c0 + ncols)
                          for (t, s_idx) in claims_by_col[cc]]
                    ns = len(cl)
                    assert 0 < ns <= SMAX, (ns, ncols)
                    s0 = cl[0][1]
                    assert [s for (_t, s, _c) in cl] == list(range(s0, s0 + ns))
                    s_tile = sp.tile([128, SMAX, 128], BF16, tag="s")
                    dstap = dst_sb[:, s0:s0 + ns].to_broadcast([128, ns, 128])
                    iap = iota_sb[:]
                    iota_b = bass.AP(iap.tensor, iap.offset,
                                     [iap.ap[0], [0, ns], iap.ap[1]])
                    nc.vector.tensor_tensor(out=s_tile[:, 0:ns, :], in0=dstap,
                                            in1=iota_b, op=mybir.AluOpType.is_equal)
                    for si, (t, s_idx, cc) in enumerate(cl):
                        B = int(bank_of_col[cc])
                        if s_idx == bank_first_s[B]:
                            acc[B] = psA.tile([128, 512], F32, tag="acc",
                                              name="acc", space="PSUM")
                        a_ps = acc[B]
                        sl = slice(128 * t, 128 * (t + 1))
                        nc.tensor.matmul(out=a_ps[:, sl],
                                         lhsT=s_tile[:, si, :],
                                         rhs=g_tile[:, cc - c0, :],
                                         start=(s_idx == bank_first_s[B]),
                                         stop=(s_idx == bank_last_s[B]))
                        if s_idx == bank_last_s[B]:
                            for t2 in range(min(BANK, NBLK - B * BANK)):
                                epilogue(B * BANK + t2,
                                         a_ps[:, 128 * t2:128 * (t2 + 1)])
                                done[0] += 1
                            del acc[B]
                assert done[0] == NBLK, done[0]

            def make_epi1(h_full):
                state = {"nxt": 0}

                def epi1(b, acc_ap):
                    t1 = ep.tile([128, HID], F32, tag="t1")
                    nc.vector.tensor_tensor(out=t1[:], in0=acc_ap,
                                            in1=y_all[:, b, :],
                                            op=mybir.AluOpType.add)
                    nc.vector.tensor_scalar_mul(out=t1[:], in0=t1[:],
                                                scalar1=dinv_sb[:, b:b + 1])
                    nc.vector.tensor_tensor(out=t1[:], in0=t1[:], in1=b1_sb[:],
                                            op=mybir.AluOpType.add)
                    nc.scalar.activation(out=h_all[:, b, :], in_=t1[:],
                                         func=mybir.ActivationFunctionType.Relu,
                                         scale=dinv_sb[:, b:b + 1])
                    while (state["nxt"] < NU
                           and b == unit_last_block[state["nxt"]]):
                        write_unit_rows(h_in, h_all, state["nxt"])
                        ag_unit(h_in, h_full, state["nxt"])
                        state["nxt"] += 1
                return epi1

            def first_half():
                y_full = alloc_full("y_full")
                phase1(y_full)
                h_full = alloc_full("h_full")
                aggregation(y_full, make_epi1(h_full))
                return h_full

            def epi2(b, acc_ap):
                t2 = ep.tile([128, HID], F32, tag="t2")
                nc.vector.tensor_tensor(out=t2[:], in0=acc_ap, in1=h_all[:, b, :],
                                        op=mybir.AluOpType.add)
                g2_sb = ep.tile([128, HID], BF16, tag="g2")
                nc.scalar.activation(out=g2_sb[:], in_=t2[:],
                                     func=mybir.ActivationFunctionType.Copy,
                                     scale=dinv_sb[:, b:b + 1])
                tp_ps = psB.tile([128, HID], BF16, tag="tp", space="PSUM")
                nc.tensor.transpose(out=tp_ps[:], in_=g2_sb[:], identity=ident_sb[:])
                g2t_sb = ep.tile([128, HID], BF16, tag="g2t")
                nc.scalar.activation(out=g2t_sb[:], in_=tp_ps[:],
                                     func=mybir.ActivationFunctionType.Copy)
                o_ps = psB.tile([128, 2 * OUT_C], F32, tag="outps", space="PSUM")
                nc.tensor.matmul(out=o_ps[:, 0:OUT_C], lhsT=g2t_sb[:], rhs=wmu_sb[:],
                                 start=True, stop=True)
                nc.tensor.matmul(out=o_ps[:, OUT_C:2 * OUT_C], lhsT=g2t_sb[:],
                                 rhs=wls_sb[:], start=True, stop=True)
                nc.vector.tensor_tensor(out=mu_all[:, OUT_C * b:OUT_C * (b + 1)],
                                        in0=o_ps[:, 0:OUT_C],
                                        in1=bmuls_sb[:, 0:OUT_C],
                                        op=mybir.AluOpType.add)
                nc.vector.tensor_tensor(out=ls_all[:, OUT_C * b:OUT_C * (b + 1)],
                                        in0=o_ps[:, OUT_C:2 * OUT_C],
                                        in1=bmuls_sb[:, OUT_C:2 * OUT_C],
                                        op=mybir.AluOpType.add)

            for _ in range(reps):
                h_full = first_half()
                aggregation(h_full, epi2)
                nc.sync.dma_start(out=mu_out.ap()[:], in_=mu_all[:])
                nc.sync.dma_start(out=ls_out.ap()[:], in_=ls_all[:])

    nc.compile()
    return nc


TUNED_CFG = {"gbufs": 12, "sbufs": 9}


def build_in_maps(inputs, dinv, per_core):
    x = np.asarray(inputs["x"])
    iota = np.tile(np.arange(128, dtype=np.float32), (128, 1)).astype(BF)
    ident = np.eye(128, dtype=np.float32).astype(BF)
    w1 = np.asarray(inputs["W1"], np.float32).astype(BF)
    wmu = np.asarray(inputs["W_mu"], np.float32).astype(BF)
    wls = np.asarray(inputs["W_ls"], np.float32).astype(BF)
    b1t = np.tile(np.asarray(inputs["b1"], np.float32), (128, 1))
    bmuls = np.tile(np.concatenate([np.asarray(inputs["b_mu"], np.float32),
                                    np.asarray(inputs["b_ls"], np.float32)]),
                    (128, 1))
    XCOLS = NBLK * 128
    in_maps = []
    for k in range(N_CORES):
        idx16, dst128 = per_core[k]
        xk = x[R * k:R * (k + 1)].astype(np.float32)
        xTk = np.zeros((128, XCOLS), dtype=BF)
        xTk[:, :R] = np.ascontiguousarray(xk.T).astype(BF)
        dv = dinv[R * k:R * (k + 1)]
        padded = np.ones(NBLK * 128, dtype=np.float32)
        padded[:R] = dv
        dinv_blk = np.ascontiguousarray(padded.reshape(NBLK, 128).T)
        in_maps.append({
            "xT": xTk, "idx_in": idx16, "dst_in": dst128, "dinv_in": dinv_blk,
            "iota_in": iota, "ident_in": ident, "w1_in": w1, "wmu_in": wmu,
            "wls_in": wls, "b1_in": b1t, "bmuls_in": bmuls,
        })
    return in_maps


def _unstage(arr):
    a = np.asarray(arr).astype(np.float32).reshape(128, NBLK, OUT_C)
    a = a.transpose(1, 0, 2)
    return np.ascontiguousarray(a.reshape(NBLK * 128, OUT_C)[:R])


def kernel(x, edge_index, W1, b1, W_mu, b_mu, W_ls, b_ls):
    inputs = {"x": x, "W1": W1, "b1": b1, "W_mu": W_mu, "b_mu": b_mu,
              "W_ls": W_ls, "b_ls": b_ls}
    dinv, plan, per_core = _prep(np.asarray(edge_index))
    nc = _build(plan, cfg=TUNED_CFG)
    in_maps = build_in_maps(inputs, dinv, per_core)
    res = bass_utils.run_bass_kernel_spmd(nc, in_maps, core_ids=list(range(N_CORES)))
    mu = np.concatenate([_unstage(res.results[k]["mu_out"]) for k in range(N_CORES)],
                        axis=0)
    ls = np.concatenate([_unstage(res.results[k]["ls_out"]) for k in range(N_CORES)],
                        axis=0)
    return (mu, ls)


# revision 12
# speedup vs baseline: 1.0825x; 1.0287x over previous
"""GCN encoder (3x GCNConv, shared mu/logstd aggregation) on 8 TRN2 NeuronCores.

Math: gcn_conv(x, A, W, b) = D^-1/2 (A+I) D^-1/2 (x W) + b; the aggregation
commutes with the right matmul:
    y~ = dinv * (x @ W1)
    h~ = relu(dinv * (dinv * (AGG y~ + y~[dst]) + b1))     (self-loop explicit)
    g2 = dinv * (AGG h~ + h~[dst])
    mu/logstd = g2 @ W_mu|W_ls + b

Sharding: nodes split contiguously across 8 cores (dst-sharded aggregation).
Source rows come from per-unit AllGather'ed full y~/h~ tables (bf16, 4 chunks
of 25000 rows = int16 index range = AG pipelining unit), fetched with
dma_gather (1024 int16 idx/call, 4 SWDGE queues). Scatter-add is one-hot
matmuls into PSUM.

Perf notes (measured): the gather is DESCRIPTOR-bound (~2.4ns/desc floor,
~56ns/desc per engine, byte-size-independent to 512B), so the stream is
packed per (bank = 4 dst blocks, chunk) cell (6% padding vs 20% for per-block
cells); boundary columns get one one-hot matmul per claiming block. Cells are
striped 4-banks-at-a-time, chunk-major inside a stripe, so gather calls form
long same-table runs and never head-of-line block on a late AllGather unit.
x is preloaded to SBUF in one DMA; self-loops are epilogue adds from
SBUF-resident y~/h~; mu/ls are staged in SBUF partition-major (bf16) and
written as two full-rate DMAs, reordered on host.

Baseline 2.87ms -> 1.55ms on 8 cores (NTFF-profiled device exec time).
"""
import numpy as np
import ml_dtypes
import concourse.bacc as bacc
import concourse.tile as tile
import concourse.bass as bass
import concourse.mybir as mybir
import concourse.bass_utils as bass_utils

N_CORES = 8
N_NODES = 100000
IN_C = 128
HID = 128
OUT_C = 64
R = N_NODES // N_CORES          # 12500 rows per core
NU = 4                          # src chunks / AllGather units
UR = R // NU                    # 3125 rows per unit per core
CHUNK = N_NODES // NU           # 25000 rows per (permuted) chunk
NBLK = (R + 127) // 128         # 98 dst blocks per core
BANK = 4                        # dst blocks per PSUM bank / stream cell
NBANK = (NBLK + BANK - 1) // BANK   # 25 banks
GCOLS = 8                       # max 128-edge cols per dma_gather (1024 idxs HW cap)
SMAX = 12                       # max one-hot claims per gather call

F32 = mybir.dt.float32
BF16 = mybir.dt.bfloat16
I16 = mybir.dt.int16
BF = ml_dtypes.bfloat16


def _wrap16(idx):
    n = idx.shape[0]
    a = idx.astype(np.int16).reshape(n // 16, 16).T
    return np.ascontiguousarray(np.tile(a, (8, 1)))


def _prep(edge_index):
    """Host prep: bank-cell packed per-core streams + shared claim schedule."""
    src = np.asarray(edge_index[0], dtype=np.int64)
    dst = np.asarray(edge_index[1], dtype=np.int64)
    deg = (np.bincount(dst, minlength=N_NODES) + 1).astype(np.float64)
    dinv = (1.0 / np.sqrt(deg)).astype(np.float32)

    k_of = src // R
    r_of = src % R
    psrc = CHUNK * (r_of // UR) + UR * k_of + (r_of % UR)
    c_of = psrc // CHUNK
    ci_of = psrc % CHUNK

    kd = dst // R
    ld = dst % R
    b_of = ld // 128
    dloc = ld % 128
    bank_of = b_of // BANK

    # cell order: stripes of 4 banks, chunk-major inside a stripe -- chunk c's
    # cells run consecutively (long same-table gather runs) and early stripes
    # only need AllGather unit c when they reach its pass (no head-of-line
    # stall on late units)
    cells = []
    for S in range((NBANK + 3) // 4):
        for c in range(NU):
            for B in range(4 * S, min(4 * S + 4, NBANK)):
                cells.append((B, c))
    rank_of = {bc: i for i, bc in enumerate(cells)}
    rank_lut = np.zeros((NBANK, NU), dtype=np.int64)
    for (B, c), i in rank_of.items():
        rank_lut[B, c] = i

    # stream order: (core, cell rank, block)
    order_key = (kd * len(cells) + rank_lut[bank_of, c_of]) * NBLK + b_of
    order = np.argsort(order_key, kind="stable")
    c_s, ci_s, kd_s, dloc_s, bank_s, b_s = (
        a[order] for a in (c_of, ci_of, kd, dloc, bank_of, b_of))
    rank_s = rank_lut[bank_s, c_s]

    cellcnt = np.zeros((N_CORES, NBANK, NU), dtype=np.int64)
    np.add.at(cellcnt, (kd_s, bank_s, c_s), 1)
    wcell = (cellcnt.max(axis=0) + 127) // 128        # [NBANK, NU] cols per cell

    blkcnt = np.zeros((N_CORES, NBANK, NU, BANK), dtype=np.int64)
    bslot = b_s - bank_s * BANK
    np.add.at(blkcnt, (kd_s, bank_s, c_s, bslot), 1)
    blkstart = np.cumsum(blkcnt, axis=3) - blkcnt      # exclusive prefix in cell
    blkend = blkstart + blkcnt

    # shared cell layout (in cell order)
    cellbase = {}
    col = 0
    for (B, c) in cells:
        if wcell[B, c] > 0:
            cellbase[(B, c)] = col
            col += int(wcell[B, c])
    LT = col
    L = LT * 128

    # shared claims per column (block slots whose [min_start, max_end) range
    # across cores touches the column)
    claims_blocks = [[] for _ in range(LT)]   # col -> [t_rel]
    for (B, c) in cells:
            if (B, c) not in cellbase:
                continue
            base = cellbase[(B, c)]
            for t in range(min(BANK, NBLK - B * BANK)):
                smin = int(blkstart[:, B, c, t].min())
                emax = int(blkend[:, B, c, t].max())
                if smin == emax:
                    continue
                for cc in range(base + smin // 128, base + (emax + 127) // 128):
                    claims_blocks[cc].append(t)
    # sequential s_idx in (column, claim) order
    claims_by_col = []
    s_counter = 0
    for cc in range(LT):
        lst = []
        for t in claims_blocks[cc]:
            lst.append((t, s_counter))
            s_counter += 1
        claims_by_col.append(lst)
    NS = s_counter

    # bank column ranges + first/last claim s_idx per bank
    bank_cols = {}
    for (B, c), base in cellbase.items():
        w = int(wcell[B, c])
        lo, hi = bank_cols.get(B, (10 ** 9, -1))
        bank_cols[B] = (min(lo, base), max(hi, base + w - 1))
    bank_first_s = {}
    bank_last_s = {}
    for B, (clo, chi) in bank_cols.items():
        ss = [s for cc in range(clo, chi + 1) for (_t, s) in claims_by_col[cc]]
        bank_first_s[B] = min(ss)
        bank_last_s[B] = max(ss)

    # gather calls: pack each consecutive same-chunk run of cells into
    # <= GCOLS-col pieces (cells of one (stripe, chunk) pass are adjacent)
    calls = []
    i = 0
    while i < len(cells):
        B0, c0c = cells[i]
        if (B0, c0c) not in cellbase:
            i += 1
            continue
        base = cellbase[(B0, c0c)]
        end = base
        while i < len(cells) and cells[i][1] == c0c and \
                cells[i] in cellbase and cellbase[cells[i]] == end:
            end += int(wcell[cells[i][0], cells[i][1]])
            i += 1
        p = base
        while p < end:
            n = min(GCOLS, end - p)
            calls.append((c0c, p, n))
            p += n

    bank_of_col = np.full(LT, -1, dtype=np.int64)
    for (B, c), base in cellbase.items():
        bank_of_col[base:base + int(wcell[B, c])] = B

    # per-core streams
    per_core = []
    for k in range(N_CORES):
        idx_arr = np.zeros(L, dtype=np.int16)
        dst_arr = np.full((NS, 128), -1.0, dtype=np.float32)
        sel = kd_s == k
        ci_k = ci_s[sel]
        c_k = c_s[sel]
        dl_k = dloc_s[sel]
        bank_k = bank_s[sel]
        b_k = b_s[sel]
        key_k = rank_lut[bank_k, c_k] * NBLK + b_k
        for (B, c), base in cellbase.items():
            rnk = rank_of[(B, c)]
            lo = np.searchsorted(key_k, rnk * NBLK)
            hi = np.searchsorted(key_k, rnk * NBLK + NBLK)
            n = hi - lo
            if n:
                idx_arr[base * 128: base * 128 + n] = ci_k[lo:hi]
            for cc in range(base, base + int(wcell[B, c])):
                p0 = lo + (cc - base) * 128
                for (t, s_idx) in claims_by_col[cc]:
                    s0 = lo + int(blkstart[k, B, c, t])
                    e0 = lo + int(blkend[k, B, c, t])
                    a0, a1 = max(s0, p0), min(e0, p0 + 128)
                    if a0 < a1:
                        dst_arr[s_idx, a0 - p0:a1 - p0] = dl_k[a0:a1]
        idx16 = _wrap16(idx_arr)
        dst128 = np.ascontiguousarray(dst_arr.T.astype(BF))   # [128, NS]
        per_core.append((idx16, dst128))

    plan = dict(calls=calls, LT=LT, L=L, NS=NS,
                claims_by_col=claims_by_col, bank_of_col=bank_of_col,
                bank_first_s=bank_first_s, bank_last_s=bank_last_s)
    return dinv, plan, per_core


def _build(plan, reps=1, nocoll=False, cfg=None):
    cfg = cfg or {}
    GBUFS = cfg.get("gbufs", 10)
    SBUFS = cfg.get("sbufs", 8)
    nc = bacc.Bacc("TRN2", target_bir_lowering=False, debug=False, num_devices=N_CORES,
                   num_swdge_queues=4,
                   dynamic_dma_scratch_size=cfg.get("dma_scratch", 32768))
    LT, L, NS = plan["LT"], plan["L"], plan["NS"]
    calls = plan["calls"]
    claims_by_col = plan["claims_by_col"]
    bank_of_col = plan["bank_of_col"]
    bank_first_s, bank_last_s = plan["bank_first_s"], plan["bank_last_s"]
    XCOLS = NBLK * 128

    xT = nc.dram_tensor("xT", [128, XCOLS], BF16, kind="ExternalInput")
    idx_in = nc.dram_tensor("idx_in", [128, L // 16], I16, kind="ExternalInput")
    dst_in = nc.dram_tensor("dst_in", [128, NS], BF16, kind="ExternalInput")
    dinv_in = nc.dram_tensor("dinv_in", [128, NBLK], F32, kind="ExternalInput")
    iota_in = nc.dram_tensor("iota_in", [128, 128], BF16, kind="ExternalInput")
    ident_in = nc.dram_tensor("ident_in", [128, 128], BF16, kind="ExternalInput")
    w1_in = nc.dram_tensor("w1_in", [128, HID], BF16, kind="ExternalInput")
    wmu_in = nc.dram_tensor("wmu_in", [HID, OUT_C], BF16, kind="ExternalInput")
    wls_in = nc.dram_tensor("wls_in", [HID, OUT_C], BF16, kind="ExternalInput")
    b1_in = nc.dram_tensor("b1_in", [128, HID], F32, kind="ExternalInput")
    bmuls_in = nc.dram_tensor("bmuls_in", [128, 2 * OUT_C], F32, kind="ExternalInput")
    mu_out = nc.dram_tensor("mu_out", [128, NBLK * OUT_C], BF16, kind="ExternalOutput")
    ls_out = nc.dram_tensor("ls_out", [128, NBLK * OUT_C], BF16, kind="ExternalOutput")

    with tile.TileContext(nc) as tc:
        with (
            tc.tile_pool(name="const", bufs=1) as cpool,
            tc.tile_pool(name="yh", bufs=1) as yhp,
            tc.tile_pool(name="gat", bufs=GBUFS) as gp,
            tc.tile_pool(name="sel", bufs=SBUFS) as sp,
            tc.tile_pool(name="epi", bufs=4) as ep,
            tc.tile_pool(name="psA", bufs=4, space="PSUM") as psA,
            tc.tile_pool(name="psB", bufs=2, space="PSUM") as psB,
            tc.tile_pool(name="dram", bufs=1, space="DRAM") as dram,
        ):
            idx_sb = cpool.tile([128, L // 16], I16)
            dst_sb = cpool.tile([128, NS], BF16)
            dinv_sb = cpool.tile([128, NBLK], F32)
            iota_sb = cpool.tile([128, 128], BF16)
            ident_sb = cpool.tile([128, 128], BF16)
            w1_sb = cpool.tile([128, HID], BF16)
            wmu_sb = cpool.tile([HID, OUT_C], BF16)
            wls_sb = cpool.tile([HID, OUT_C], BF16)
            b1_sb = cpool.tile([128, HID], F32)
            bmuls_sb = cpool.tile([128, 2 * OUT_C], F32)
            for sb, dr in ((idx_sb, idx_in), (dst_sb, dst_in), (dinv_sb, dinv_in),
                           (iota_sb, iota_in), (ident_sb, ident_in), (w1_sb, w1_in),
                           (wmu_sb, wmu_in), (wls_sb, wls_in), (b1_sb, b1_in),
                           (bmuls_sb, bmuls_in)):
                nc.sync.dma_start(out=sb[:], in_=dr.ap()[:])

            x_all = yhp.tile([128, XCOLS], BF16, tag="xbig", name="x_all")
            nc.sync.dma_start(out=x_all[:], in_=xT.ap()[:])
            y_all = cpool.tile([128, NBLK, 128], BF16, name="y_all")
            h_all = cpool.tile([128, NBLK, 128], BF16, name="h_all")
            mu_all = cpool.tile([128, NBLK * OUT_C], BF16, name="mu_all")
            ls_all = cpool.tile([128, NBLK * OUT_C], BF16, name="ls_all")

            y_in = dram.tile([R, HID], BF16)
            h_in = dram.tile([R, HID], BF16)

            def alloc_full(pfx):
                return [dram.tile([CHUNK, HID], BF16, addr_space="Shared",
                                  tag=f"{pfx}{j}", name=f"{pfx}{j}")
                        for j in range(NU)]

            unit_last_block = [(UR * (j + 1) - 1) // 128 for j in range(NU)]

            def write_unit_rows(dram_t, sb_all, j):
                lo, hi = UR * j, UR * (j + 1)
                b0, p0 = lo // 128, lo % 128
                b1, p1 = hi // 128, hi % 128
                if p0:
                    n = 128 - p0
                    nc.sync.dma_start(out=dram_t[lo:lo + n, :],
                                      in_=sb_all[p0:p0 + n, b0, :])
                    lo += n
                    b0 += 1
                if b0 < b1:
                    nc.sync.dma_start(
                        out=dram_t[128 * b0:128 * b1, :].rearrange(
                            "(b p) f -> p b f", p=128),
                        in_=sb_all[:, b0:b1, :])
                if p1:
                    nc.sync.dma_start(out=dram_t[128 * b1:128 * b1 + p1, :],
                                      in_=sb_all[0:p1, b1, :])

            def ag_unit(t_in, t_full, j):
                if nocoll:
                    nc.sync.dma_start(out=t_full[j][0:UR, :],
                                      in_=t_in[UR * j:UR * (j + 1), :])
                else:
                    nc.gpsimd.collective_compute(
                        "AllGather", mybir.AluOpType.bypass,
                        replica_groups=[list(range(N_CORES))],
                        ins=[t_in[UR * j:UR * (j + 1), :]],
                        outs=[t_full[j][:]],
                    )

            def phase1(y_full):
                nxt = 0
                for b in range(NBLK):
                    y_ps = psB.tile([128, HID], F32, tag="outps", space="PSUM",
                                    name="y_ps")
                    nc.tensor.matmul(out=y_ps[:], lhsT=x_all[:, 128 * b:128 * (b + 1)],
                                     rhs=w1_sb[:], start=True, stop=True)
                    nc.scalar.activation(out=y_all[:, b, :], in_=y_ps[:],
                                         func=mybir.ActivationFunctionType.Copy,
                                         scale=dinv_sb[:, b:b + 1])
                    while nxt < NU and b == unit_last_block[nxt]:
                        write_unit_rows(y_in, y_all, nxt)
                        ag_unit(y_in, y_full, nxt)
                        nxt += 1

            def aggregation(table, epilogue):
                acc = {}
                done = [0]
                for ci, (c, c0, ncols) in enumerate(calls):
                    g_tile = gp.tile([128, GCOLS, 128], BF16, tag="g")
                    nc.gpsimd.dma_gather(
                        out_ap=g_tile[:, 0:ncols, :],
                        in_ap=table[c][:],
                        idxs_ap=idx_sb[:, 8 * c0: 8 * (c0 + ncols)],
                        num_idxs=128 * ncols, num_idxs_reg=128 * ncols,
                        elem_size=128,
                        queue_num=ci % 4,
                        single_packet=True,
                    )
                    cl = [(t, s_idx, cc) for cc in range(c0, c0 + ncols)
                          for (t, s_idx) in claims_by_col[cc]]
                    ns = len(cl)
                    assert 0 < ns <= SMAX, (ns, ncols)
                    s0 = cl[0][1]
                    assert [s for (_t, s, _c) in cl] == list(range(s0, s0 + ns))
                    s_tile = sp.tile([128, SMAX, 128], BF16, tag="s")
                    dstap = dst_sb[:, s0:s0 + ns].to_broadcast([128, ns, 128])
                    iap = iota_sb[:]
                    iota_b = bass.AP(iap.tensor, iap.offset,
                                     [iap.ap[0], [0, ns], iap.ap[1]])
                    nc.vector.tensor_tensor(out=s_tile[:, 0:ns, :], in0=dstap,
                                            in1=iota_b, op=mybir.AluOpType.is_equal)
                    for si, (t, s_idx, cc) in enumerate(cl):
                        B = int(bank_of_col[cc])
                        if s_idx == bank_first_s[B]:
                            acc[B] = psA.tile([128, 512], F32, tag="acc",
                                              name="acc", space="PSUM")
                        a_ps = acc[B]
                        sl = slice(128 * t, 128 * (t + 1))
                        nc.tensor.matmul(out=a_ps[:, sl],
                                         lhsT=s_tile[:, si, :],
                                         rhs=g_tile[:, cc - c0, :],
                                         start=(s_idx == bank_first_s[B]),
                                         stop=(s_idx == bank_last_s[B]))
                        if s_idx == bank_last_s[B]:
                            for t2 in range(min(BANK, NBLK - B * BANK)):
                                epilogue(B * BANK + t2,
                                         a_ps[:, 128 * t2:128 * (t2 + 1)])
                                done[0] += 1
                            del acc[B]
                assert done[0] == NBLK, done[0]

            def make_epi1(h_full):
                state = {"nxt": 0}

                def epi1(b, acc_ap):
                    t1 = ep.tile([128, HID], F32, tag="t1")
                    nc.vector.tensor_tensor(out=t1[:], in0=acc_ap,
                                            in1=y_all[:, b, :],
                                            op=mybir.AluOpType.add)
                    nc.vector.tensor_scalar_mul(out=t1[:], in0=t1[:],
                                                scalar1=dinv_sb[:, b:b + 1])
                    nc.vector.tensor_tensor(out=t1[:], in0=t1[:], in1=b1_sb[:],
                                            op=mybir.AluOpType.add)
                    nc.scalar.activation(out=h_all[:, b, :], in_=t1[:],
                                         func=mybir.ActivationFunctionType.Relu,
                                         scale=dinv_sb[:, b:b + 1])
                    while (state["nxt"] < NU
                           and b == unit_last_block[state["nxt"]]):
                        write_unit_rows(h_in, h_all, state["nxt"])
                        ag_unit(h_in, h_full, state["nxt"])
                        state["nxt"] += 1
                return epi1

            def first_half():
                y_full = alloc_full("y_full")
                phase1(y_full)
                h_full = alloc_full("h_full")
                aggregation(y_full, make_epi1(h_full))
                return h_full

            def epi2(b, acc_ap):
                t2 = ep.tile([128, HID], F32, tag="t2")
                nc.vector.tensor_tensor(out=t2[:], in0=acc_ap, in1=h_all[:, b, :],
                                        op=mybir.AluOpType.add)
                g2_sb = ep.tile([128, HID], BF16, tag="g2")
                nc.scalar.activation(out=g2_sb[:], in_=t2[:],
                                     func=mybir.ActivationFunctionType.Copy,
                                     scale=dinv_sb[:, b:b + 1])
                tp_ps = psB.tile([128, HID], BF16, tag="tp", space="PSUM")
                nc.tensor.transpose(out=tp_ps[:], in_=g2_sb[:], identity=ident_sb[:])
                g2t_sb = ep.tile([128, HID], BF16, tag="g2t")
                nc.scalar.activation(out=g2t_sb[:], in_=tp_ps[:],
                                     func=mybir.ActivationFunctionType.Copy)
                o_ps = psB.tile([128, 2 * OUT_C], F32, tag="outps", space="PSUM")
                nc.tensor.matmul(out=o_ps[:, 0:OUT_C], lhsT=g2t_sb[:], rhs=wmu_sb[:],
                                 start=True, stop=True)
                nc.tensor.matmul(out=o_ps[:, OUT_C:2 * OUT_C], lhsT=g2t_sb[:],
                                 rhs=wls_sb[:], start=True, stop=True)
                nc.vector.tensor_tensor(out=mu_all[:, OUT_C * b:OUT_C * (b + 1)],
                                        in0=o_ps[:, 0:OUT_C],
                                        in1=bmuls_sb[:, 0:OUT_C],
                                        op=mybir.AluOpType.add)
                nc.vector.tensor_tensor(out=ls_all[:, OUT_C * b:OUT_C * (b + 1)],
                                        in0=o_ps[:, OUT_C:2 * OUT_C],
                                        in1=bmuls_sb[:, OUT_C:2 * OUT_C],
                                        op=mybir.AluOpType.add)

            for _ in range(reps):
                h_full = first_half()
                aggregation(h_full, epi2)
                nc.sync.dma_start(out=mu_out.ap()[:], in_=mu_all[:])
                nc.sync.dma_start(out=ls_out.ap()[:], in_=ls_all[:])

    nc.compile()
    return nc


TUNED_CFG = {"gbufs": 10, "sbufs": 8}


def build_in_maps(inputs, dinv, per_core):
    x = np.asarray(inputs["x"])
    iota = np.tile(np.arange(128, dtype=np.float32), (128, 1)).astype(BF)
    ident = np.eye(128, dtype=np.float32).astype(BF)
    w1 = np.asarray(inputs["W1"], np.float32).astype(BF)
    wmu = np.asarray(inputs["W_mu"], np.float32).astype(BF)
    wls = np.asarray(inputs["W_ls"], np.float32).astype(BF)
    b1t = np.tile(np.asarray(inputs["b1"], np.float32), (128, 1))
    bmuls = np.tile(np.concatenate([np.asarray(inputs["b_mu"], np.float32),
                                    np.asarray(inputs["b_ls"], np.float32)]),
                    (128, 1))
    XCOLS = NBLK * 128
    in_maps = []
    for k in range(N_CORES):
        idx16, dst128 = per_core[k]
        xk = x[R * k:R * (k + 1)].astype(np.float32)
        xTk = np.zeros((128, XCOLS), dtype=BF)
        xTk[:, :R] = np.ascontiguousarray(xk.T).astype(BF)
        dv = dinv[R * k:R * (k + 1)]
        padded = np.ones(NBLK * 128, dtype=np.float32)
        padded[:R] = dv
        dinv_blk = np.ascontiguousarray(padded.reshape(NBLK, 128).T)
        in_maps.append({
            "xT": xTk, "idx_in": idx16, "dst_in": dst128, "dinv_in": dinv_blk,
            "iota_in": iota, "ident_in": ident, "w1_in": w1, "wmu_in": wmu,
            "wls_in": wls, "b1_in": b1t, "bmuls_in": bmuls,
        })
    return in_maps


def _unstage(arr):
    a = np.asarray(arr).astype(np.float32).reshape(128, NBLK, OUT_C)
    a = a.transpose(1, 0, 2)
    return np.ascontiguousarray(a.reshape(NBLK * 128, OUT_C)[:R])


def kernel(x, edge_index, W1, b1, W_mu, b_mu, W_ls, b_ls):
    inputs = {"x": x, "W1": W1, "b1": b1, "W_mu": W_mu, "b_mu": b_mu,
              "W_ls": W_ls, "b_ls": b_ls}
    dinv, plan, per_core = _prep(np.asarray(edge_index))
    nc = _build(plan, cfg=TUNED_CFG)
    in_maps = build_in_maps(inputs, dinv, per_core)
    res = bass_utils.run_bass_kernel_spmd(nc, in_maps, core_ids=list(range(N_CORES)))
    mu = np.concatenate([_unstage(res.results[k]["mu_out"]) for k in range(N_CORES)],
                        axis=0)
    ls = np.concatenate([_unstage(res.results[k]["ls_out"]) for k in range(N_CORES)],
                        axis=0)
    return (mu, ls)
